# revision 30
# baseline (speedup 1.0000x reference)
"""Trainium2 Bass kernel for nn_MultiHeadAttention_3796751090171 (sparse_attention).

Batch-parallel SPMD across 8 NeuronCores: q_batch/k_batch are SORTED, so the
cross-batch mask makes attention block-diagonal over batches, and there are
exactly B=8 batches for 8 cores. Core c computes batch c's queries against
batch c's keys for ALL 8 heads -- completely independent work, NO collectives.

v2 redesign vs the 512x512-padded baseline (87.4us -> ~57us):
  - Exact tile sizes: NQ = max_c nq_c (multiple of 4), NK = max_c nk_c; the
    last k-chunk is partial (NKR rows); pad state zeroed once on-chip
    (KT_f pad columns, V_sb pad rows) or host-side (pos pad rows).
  - All inputs are pre-arranged by the host into their exact SBUF layouts,
    so every load is one contiguous hardware-DGE DMA issued at t=0
    (features+pos chunks on the SP queue, weights on the Act queue) instead
    of ~65 per-tile engine-blocking DMA triggers. exp(pos) tiles stay
    resident in SBUF (~2.7 MB).
  - scoresT matmuls (contraction = head_dim = 64) are ROW-PACKED: the two
    heads of a pair run concurrently on PE row-groups 0-1/2-3 via
    tile_position, overlapping each other's LDWEIGHTS.
  - One ACT exp per (head, kc-pair) across two PSUM banks (16 x ~1us instead
    of 32 x 0.6us); the exp(pos) multiply runs on DVE in 2x bf16 mode (pos
    layout keeps kc-pairs contiguous).
  - V carries 64 ones-columns so stage2 lands Z replicated on psum rows
    64:128: normalization is copy+reciprocal_approx+multiply on DVE only --
    no cross-partition broadcast (no DRAM bounce, no extra matmul).
  - V is projected directly into [k, d] layout (no PE transposes).
  - K/Q projections are interleaved with the attention head-pair loop and a
    ~2.5us identity-matmul warmup covers the initial DMA wait, keeping the
    PE HAM clock-gate at 2.4 GHz.

Per core c (batch slice qs:qe / ks:ke, all heads h):
  Q^T = Wq^T/8 @ qf^T, K^T = Wk^T @ kf^T   ([d, n] layouts)
  V   = vf @ Wv  in [k, (h, d|ones)] layout (ones cols -> Z rows in stage2)
  per (h, kc): scoresT[k,q] = K_h^T-chunk @ Q_h (PSUM); expt = exp(scoresT)
    * posc[h,kc]  (posc holds exp(pos); 0 on pad-k rows, 1 on pad-q cols)
  stage2: [hT; Z..Z] += [V|1s]^T-chunk @ expt ; hTn = hT * approx(1/Z)
  outT[o,q] = sum_t Wo[128t:,oc].T @ hTn_t
Host: out[qs:qe, :] = outT[:, :nq].T
"""

import functools
import math

import numpy as np
import ml_dtypes

import concourse.bass as bass
import concourse.tile as tile
from concourse import bacc, mybir
from concourse.bass_utils import run_bass_kernel_spmd
from concourse.masks import make_identity

N = 3072
QD = 512
OD = 512
H = 8
D = 64
B = 8
NCORES = 8
SCALE = math.sqrt(D)
KT_T = QD // 128   # contraction tiles for the projections
NTD = OD // 128    # output-d tiles

F32 = mybir.dt.float32
BF16 = mybir.dt.bfloat16
BF16_NP = ml_dtypes.bfloat16

TRACE = False
LAST_RESULTS = None


def _bounds(q_batch, k_batch):
    qb = np.asarray(q_batch).astype(np.int64)
    kb = np.asarray(k_batch).astype(np.int64)
    qbound = np.searchsorted(qb, np.arange(B + 1))
    kbound = np.searchsorted(kb, np.arange(B + 1))
    return qbound, kbound


@functools.lru_cache(maxsize=8)
def _build(NQ, NK, has_bq, has_bk, has_bv, has_bo):
    assert NQ <= 512 and NK <= 512
    nc = bacc.Bacc("TRN2", target_bir_lowering=False, debug=False,
                   num_devices=NCORES)

    NKC = (NK + 127) // 128          # k chunks (last may be partial)
    KPAD = NKC * 128
    NKR = NK - 128 * (NKC - 1)       # rows in the last chunk
    NKF = NKC - 1                    # number of full chunks

    # all inputs pre-arranged by the host into their SBUF layouts, so each
    # DMA is a contiguous per-partition blit
    qfT_d = nc.dram_tensor("qfT", [128, KT_T * NQ], BF16, kind="ExternalInput")
    kfT_d = nc.dram_tensor("kfT", [128, KT_T * NK], BF16, kind="ExternalInput")
    vfT_d = nc.dram_tensor("vfT", [128, KT_T * NK], BF16, kind="ExternalInput")
    posc_d = nc.dram_tensor("posc", [128, (H // 2) * NKC * 2 * NQ], BF16,
                            kind="ExternalInput")
    wq_d = nc.dram_tensor("wq", [128, KT_T * OD], BF16, kind="ExternalInput")
    wk_d = nc.dram_tensor("wk", [128, KT_T * OD], BF16, kind="ExternalInput")
    wv_d = nc.dram_tensor("wv", [128, KT_T * OD], BF16, kind="ExternalInput")
    wo_d = nc.dram_tensor("wo", [128, NTD * NTD * 128], BF16, kind="ExternalInput")
    bq_d = nc.dram_tensor("bq", [1, OD], BF16, kind="ExternalInput") if has_bq else None
    bk_d = nc.dram_tensor("bk", [1, OD], BF16, kind="ExternalInput") if has_bk else None
    bv_d = nc.dram_tensor("bv", [1, OD], BF16, kind="ExternalInput") if has_bv else None
    bo_d = nc.dram_tensor("bo", [128, NTD], F32, kind="ExternalInput") if has_bo else None
    out_d = nc.dram_tensor("out", [OD, NQ], BF16, kind="ExternalOutput")
    import os
    DEBUG = bool(os.environ.get("KDBG"))
    if DEBUG:
        dbg_z = nc.dram_tensor("dbg_z", [D, NQ], F32, kind="ExternalOutput")
        dbg_zr = nc.dram_tensor("dbg_zr", [D, NQ], F32, kind="ExternalOutput")
        dbg_htn = nc.dram_tensor("dbg_htn", [128, NTD, NQ], F32, kind="ExternalOutput")
        dbg_v = nc.dram_tensor("dbg_v", [128, NKC, H, 128], F32, kind="ExternalOutput")

    with tile.TileContext(nc) as tc:
        with (
            tc.tile_pool(name="consts", bufs=1) as consts,
            tc.tile_pool(name="expp", bufs=8) as expp,
            tc.tile_pool(name="outp", bufs=2) as outp,
            tc.tile_pool(name="zrp", bufs=2) as zrp,
            tc.tile_pool(name="ps_s", bufs=2, space="PSUM") as ps_s,
            tc.tile_pool(name="ps_h", bufs=2, space="PSUM") as ps_h,
            tc.tile_pool(name="ps_p", bufs=2, space="PSUM") as ps_p,
            tc.tile_pool(name="dram", bufs=1, space="DRAM") as dramp,
        ):
            # ---------------- constants / one-time setup ----------------
            ones = consts.tile([1, max(NQ, NK, OD)], BF16)
            nc.vector.memset(ones, 1.0)
            ident128 = consts.tile([128, 128], BF16)
            make_identity(nc, ident128)

            # feature tiles [128, t, n]
            qf_sb = consts.tile([128, KT_T, NQ], BF16)
            kf_sb = consts.tile([128, KT_T, NK], BF16)
            vf_sb = consts.tile([128, KT_T, NK], BF16)
            # weights [128, t, od]; wo as [128, t, oc, 128]
            wq_sb = consts.tile([128, KT_T, OD], BF16)
            wk_sb = consts.tile([128, KT_T, OD], BF16)
            wv_sb = consts.tile([128, KT_T, OD], BF16)
            wo_sb = consts.tile([128, NTD, NTD, 128], BF16)
            # exp(pos) resident tiles: [128, hpair, h%2, kc, q]
            # (kc innermost so a kc-pair slice is contiguous -> DVE 2x mode)
            pos_sb = consts.tile([128, H // 2, 2, NKC, NQ], BF16)
            # projected tensors
            QT_f = consts.tile([128, NTD, NQ], BF16, name="QT_f")
            KT_f = consts.tile([128, NTD, KPAD], BF16, name="KT_f")
            V_sb = consts.tile([128, NKC, H, 128], BF16, name="V_sb")
            hTn_sb = consts.tile([128, NTD, NQ], BF16, name="hTn_sb")

            # force the ACT exp-table load now (one-time ~1.3us) instead of
            # at the first real exp
            dummy = consts.tile([1, 2], F32, name="dummy")
            nc.vector.memset(dummy, 1.0)
            nc.scalar.activation(dummy[0:1, 1:2], dummy[0:1, 0:1],
                                 mybir.ActivationFunctionType.Exp)

            # zero-pad state for the partial last k chunk (pos pad rows are
            # zeroed host-side). Engine APs must start at partition 0/32/64/96,
            # so zero the whole last-chunk slice; real rows are written later.
            if NK < KPAD:
                nc.vector.memset(KT_f[:, :, NK:KPAD], 0.0)
                nc.gpsimd.memset(V_sb[:, NKC - 1, :, :], 0.0)
            # ones columns of V (-> Z on psum rows 64:128); real k rows only
            if NKF > 0:
                nc.gpsimd.memset(V_sb[:, 0:NKF, :, D:128], 1.0)
            nc.gpsimd.memset(V_sb[0:NKR, NKC - 1, :, D:128], 1.0)

            # HAM warmup: dummy matmuls while the first DMAs land, so the
            # projections start at 2.4 GHz instead of 1.2
            warm_ps = ps_p.tile([128, 512], F32, tag="psp")
            for wi in range(24):
                nc.tensor.matmul(warm_ps[:, 0:128], ident128[:, :],
                                 ident128[:, :], start=(wi == 0), stop=(wi == 23))
            warm_sb = consts.tile([1, 1], F32, name="warm_sb")
            nc.vector.tensor_copy(warm_sb[0:1, 0:1], warm_ps[0:1, 0:1])
            warm_d = dramp.tile([1, 1], F32)
            nc.gpsimd.dma_start(out=warm_d[:, :], in_=warm_sb[0:1, 0:1])

            # ---------------- bulk DMAs (HW DGE queues) ----------------
            # SP queue: features then pos (in consumption order)
            nc.sync.dma_start(out=vf_sb, in_=vfT_d[:, :])
            nc.sync.dma_start(out=kf_sb, in_=kfT_d[:, :])
            # pos in chunks so each (hp, kc) tile unblocks its consumers
            # as soon as it lands
            for hp in range(H // 2):
                for hh in range(2):
                    off = (hp * 2 + hh) * NKC * NQ
                    nc.sync.dma_start(out=pos_sb[:, hp, hh, :, :],
                                      in_=posc_d[:, off:off + NKC * NQ])
            # Act queue: weights
            nc.scalar.dma_start(out=wv_sb, in_=wv_d[:, :])
            nc.scalar.dma_start(out=wk_sb, in_=wk_d[:, :])
            nc.scalar.dma_start(out=wq_sb, in_=wq_d[:, :])
            nc.scalar.dma_start(out=qf_sb, in_=qfT_d[:, :])
            nc.scalar.dma_start(out=wo_sb, in_=wo_d[:, :])
            bias_sb = {}
            for nm, dd in (("bq", bq_d), ("bk", bk_d), ("bv", bv_d)):
                if dd is not None:
                    t = consts.tile([1, OD], BF16, tag=f"bias_{nm}", name=f"b_{nm}")
                    nc.scalar.dma_start(out=t, in_=dd[:, :])
                    bias_sb[nm] = t
            if bo_d is not None:
                bo_sb = consts.tile([128, NTD], F32)
                nc.scalar.dma_start(out=bo_sb, in_=bo_d[:, :])

            # ---------------- V projection: [k, (h, d)] layout ----------------
            for kc in range(NKC):
                m = 128 if kc < NKF else NKR
                ksl = slice(128 * kc, 128 * kc + m)
                psum = ps_p.tile([128, 512], F32, tag="psp")
                for t in range(KT_T):
                    nc.tensor.matmul(psum[0:m, 0:OD], vf_sb[:, t, ksl],
                                     wv_sb[:, t, :], start=(t == 0),
                                     stop=(t == KT_T - 1 and "bv" not in bias_sb))
                if "bv" in bias_sb:
                    # += ones_k^T (x) bv : K=1 outer product adds bv per row
                    nc.tensor.matmul(psum[0:m, 0:OD], ones[0:1, 0:m],
                                     bias_sb["bv"][0:1, :], start=False, stop=True)
                nc.vector.tensor_copy(V_sb[0:m, kc, :, 0:D], psum[0:m, 0:OD])

            # ---------------- K/Q projections (per output-d tile) ----------
            def proj_td(td, which):
                dsl = slice(128 * td, 128 * (td + 1))
                if which == "k":
                    f_sb, w_sb, dst, nn, bias = kf_sb, wk_sb, KT_f, NK, bias_sb.get("bk")
                else:
                    f_sb, w_sb, dst, nn, bias = qf_sb, wq_sb, QT_f, NQ, bias_sb.get("bq")
                psum = ps_p.tile([128, 512], F32, tag="psp")
                for t in range(KT_T):
                    nc.tensor.matmul(psum[:, 0:nn], w_sb[:, t, dsl],
                                     f_sb[:, t, 0:nn], start=(t == 0),
                                     stop=(t == KT_T - 1 and bias is None))
                if bias is not None:
                    nc.tensor.matmul(psum[:, 0:nn], bias[:, dsl],
                                     ones[:, 0:nn], start=False, stop=True)
                nc.vector.tensor_copy(dst[:, td, 0:nn], psum[:, 0:nn])

            proj_td(0, "k")
            proj_td(0, "q")
            early_oc = {}

            # ---------------- attention: head-pair loop ----------------
            def scores_pair(hp, k0, kw):
                """Row-packed scoresT for both heads of the pair over kc in
                [k0, k0+kw): head 2hp on PE rows 0:64, head 2hp+1 on rows
                64:128 -- the two matmuls run concurrently and each head's
                LDWEIGHTS overlaps the other head's matmul. Then one exp +
                one exp(pos)-multiply per head. Returns the two expt tiles."""
                pss = [ps_s.tile([128, 2, 512], F32, tag="pss", name="ps0"),
                       ps_s.tile([128, 2, 512], F32, tag="pss", name="ps1")]
                for j in range(kw):
                    ksl = slice(128 * (k0 + j), 128 * (k0 + j + 1))
                    for hh in range(2):
                        psl = slice(D * hh, D * (hh + 1))
                        nc.tensor.matmul(pss[hh][:, j, 0:NQ],
                                         KT_f[psl, hp, ksl],
                                         QT_f[psl, hp, 0:NQ],
                                         start=True, stop=True,
                                         tile_position=(D * hh, 0))
                out = []
                for hh in range(2):
                    expr = expp.tile([128, 2, NQ], BF16, tag="expr")
                    nc.scalar.activation(expr[:, 0:kw, :],
                                         pss[hh][:, 0:kw, 0:NQ],
                                         mybir.ActivationFunctionType.Exp)
                    expt = expp.tile([128, 2, NQ], BF16, tag="expt")
                    nc.vector.tensor_tensor(out=expt[:, 0:kw, :],
                                            in0=expr[:, 0:kw, :],
                                            in1=pos_sb[:, hp, hh, k0:k0 + kw, :],
                                            op=mybir.AluOpType.mult)
                    out.append(expt)
                return out

            def stage2(h, expts):
                psum_h = ps_h.tile([128, NQ], F32, tag="psh")
                for kc in range(NKC):
                    nc.tensor.matmul(psum_h[:, :], V_sb[:, kc, h, :],
                                     expts[kc // 2][:, kc % 2, :],
                                     start=(kc == 0), stop=(kc == NKC - 1))
                return psum_h

            def norm(h, psum_h):
                # Z sits replicated on psum rows 64:128 (ones columns of V);
                # 1/Z straight off PSUM, then scale hT
                hp, hh = h // 2, h % 2
                po = D * hh
                zsb = zrp.tile([D, NQ], F32, tag="zsb")
                nc.scalar.copy(zsb[:, :], psum_h[D:128, :])
                zrec = zrp.tile([D, NQ], F32, tag="zrb")
                nc.vector.reciprocal_approx_fast(zrec[:, :], zsb[:, :])
                if DEBUG and h == 0:
                    nc.sync.dma_start(out=dbg_z.ap(), in_=zsb[:, :])
                    nc.sync.dma_start(out=dbg_zr.ap(), in_=zrec[:, :])
                nc.vector.tensor_tensor(out=hTn_sb[po:po + D, hp, 0:NQ],
                                        in0=psum_h[0:D, :], in1=zrec[:, :],
                                        op=mybir.AluOpType.mult)

            for hp in range(H // 2):
                expts = {0: [], 1: []}
                for k0 in range(0, NKC, 2):
                    kw = min(2, NKC - k0)
                    e0, e1 = scores_pair(hp, k0, kw)
                    expts[0].append(e0)
                    expts[1].append(e1)
                # next head-pair's projections keep the PE busy while the
                # ACT/DVE/Pool chain chews on this pair's exp tiles
                if hp + 1 < H // 2:
                    proj_td(hp + 1, "k")
                    proj_td(hp + 1, "q")
                if hp == H // 2 - 1:
                    # pre-accumulate outproj over the already-finished hTn
                    # slots (t=0..2) -- fills the PE wait on this pair's
                    # exp/stage2 chain and shrinks the tail
                    for oc in range(2):
                        psum = ps_p.tile([128, 512], F32, tag="psp",
                                         name=f"oc{oc}")
                        for t in range(NTD - 1):
                            nc.tensor.matmul(psum[:, 0:NQ],
                                             wo_sb[:, t, oc, :],
                                             hTn_sb[:, t, 0:NQ],
                                             start=(t == 0), stop=False)
                        early_oc[oc] = psum
                ph0 = stage2(2 * hp, expts[0])
                ph1 = stage2(2 * hp + 1, expts[1])
                norm(2 * hp, ph0)
                norm(2 * hp + 1, ph1)

            if DEBUG:
                vdbg = consts.tile([128, NKC, H, 128], F32, name="vdbg")
                nc.vector.tensor_copy(vdbg[:, :, :, :], V_sb[:, :, :, :])
                nc.sync.dma_start(out=dbg_v.ap(), in_=vdbg[:, :, :, :])
                hdbg = consts.tile([128, NTD, NQ], F32, name="hdbg")
                nc.vector.tensor_copy(hdbg[:, :, :], hTn_sb[:, :, :])
                nc.sync.dma_start(out=dbg_htn.ap(), in_=hdbg[:, :, :])

            # ---------------- output projection ----------------
            for oc in range(NTD):
                if oc in early_oc:
                    psum = early_oc[oc]
                    nc.tensor.matmul(psum[:, 0:NQ], wo_sb[:, NTD - 1, oc, :],
                                     hTn_sb[:, NTD - 1, 0:NQ],
                                     start=False, stop=True)
                else:
                    psum = ps_p.tile([128, 512], F32, tag="psp")
                    for t in range(NTD):
                        nc.tensor.matmul(psum[:, 0:NQ], wo_sb[:, t, oc, :],
                                         hTn_sb[:, t, 0:NQ],
                                         start=(t == 0), stop=(t == NTD - 1))
                o_sb = outp.tile([128, NQ], BF16, tag="osb")
                if bo_d is not None:
                    nc.scalar.activation(o_sb[:, :], psum[:, 0:NQ],
                                         mybir.ActivationFunctionType.Identity,
                                         bias=bo_sb[:, oc:oc + 1])
                else:
                    nc.scalar.copy(o_sb[:, :], psum[:, 0:NQ])
                nc.sync.dma_start(out=out_d[128 * oc:128 * (oc + 1), :],
                                  in_=o_sb[:, :])

    nc.compile()
    return nc


def _kernel_numpy(q_feat, k_feat, v_feat, pos_enc, Wq, bq, Wk, bk, Wv, bv,
                  Wo, bo, q_batch, k_batch):
    """Host fallback (degenerate batch layouts) + debugging aid."""
    Q = (q_feat @ Wq + bq).reshape(N, H, D).transpose(1, 0, 2)
    K = (k_feat @ Wk + bk).reshape(N, H, D).transpose(1, 0, 2)
    V = (v_feat @ Wv + bv).reshape(N, H, D).transpose(1, 0, 2)
    scores = np.einsum("hnd,hmd->hnm", Q, K) / SCALE + pos_enc
    mask = q_batch[:, None] != k_batch[None, :]
    scores = np.where(mask[None], np.float32(-1e9), scores)
    scores = scores - scores.max(-1, keepdims=True)
    e = np.exp(scores)
    probs = e / e.sum(-1, keepdims=True)
    h = np.einsum("hnm,hmd->hnd", probs, V)
    h = h.transpose(1, 0, 2).reshape(N, OD)
    return (h @ Wo + bo).astype(np.float32)


def kernel(q_feat, k_feat, v_feat, pos_enc, Wq, bq, Wk, bk, Wv, bv, Wo, bo,
           q_batch, k_batch):
    global LAST_RESULTS
    args = dict(q_feat=np.asarray(q_feat, np.float32),
                k_feat=np.asarray(k_feat, np.float32),
                v_feat=np.asarray(v_feat, np.float32),
                pos_enc=np.asarray(pos_enc, np.float32),
                Wq=np.asarray(Wq, np.float32), bq=np.asarray(bq, np.float32),
                Wk=np.asarray(Wk, np.float32), bk=np.asarray(bk, np.float32),
                Wv=np.asarray(Wv, np.float32), bv=np.asarray(bv, np.float32),
                Wo=np.asarray(Wo, np.float32), bo=np.asarray(bo, np.float32),
                q_batch=np.asarray(q_batch), k_batch=np.asarray(k_batch))

    qbound, kbound = _bounds(args["q_batch"], args["k_batch"])
    nq_all = np.diff(qbound)
    nk_all = np.diff(kbound)
    if np.any((nq_all > 0) & (nk_all == 0)) or nq_all.max() > 512 \
            or nk_all.max() > 512:
        # a batch with queries but no keys (reference -> uniform attention
        # over ALL keys), or tiles beyond the single-chunk design: fall back
        return _kernel_numpy(**args)

    NQ = (int(nq_all.max()) + 3) // 4 * 4
    NK = (int(nk_all.max()) + 1) // 2 * 2

    has_bq = bool(np.any(args["bq"]))
    has_bk = bool(np.any(args["bk"]))
    has_bv = bool(np.any(args["bv"]))
    has_bo = bool(np.any(args["bo"]))

    nc = _build(NQ, NK, has_bq, has_bk, has_bv, has_bo)

    # ---- host-side sharding / layout / padding ----
    NKC = (NK + 127) // 128
    KPAD = NKC * 128

    def feat_tiles(x):
        # [QD, n] -> [128, KT_T * n] (SBUF layout [p, t, n])
        return np.ascontiguousarray(
            x.reshape(KT_T, 128, -1).transpose(1, 0, 2).reshape(128, -1))

    def w_tiles(w):
        # [QD, OD] -> [128, KT_T * OD]
        return np.ascontiguousarray(
            w.reshape(KT_T, 128, OD).transpose(1, 0, 2).reshape(128, -1))

    qfT = np.ascontiguousarray(args["q_feat"].T).astype(BF16_NP)
    kfT = np.ascontiguousarray(args["k_feat"].T).astype(BF16_NP)
    vfT = np.ascontiguousarray(args["v_feat"].T).astype(BF16_NP)
    wq8 = w_tiles((args["Wq"] / SCALE).astype(BF16_NP))
    wkb = w_tiles(args["Wk"].astype(BF16_NP))
    wvb = w_tiles(args["Wv"].astype(BF16_NP))
    # [OD, OD] -> [128, (t, oc, 128)]
    wob = np.ascontiguousarray(
        args["Wo"].astype(BF16_NP).reshape(NTD, 128, NTD, 128)
        .transpose(1, 0, 2, 3).reshape(128, -1))

    in_maps = []
    for c in range(NCORES):
        qs, qe = int(qbound[c]), int(qbound[c + 1])
        ks, ke = int(kbound[c]), int(kbound[c + 1])
        nq, nk = qe - qs, ke - ks

        qfc = np.zeros((QD, NQ), BF16_NP)
        qfc[:, :nq] = qfT[:, qs:qe]
        kfc = np.zeros((QD, NK), BF16_NP)
        kfc[:, :nk] = kfT[:, ks:ke]
        vfc = np.zeros((QD, NK), BF16_NP)
        vfc[:, :nk] = vfT[:, ks:ke]

        # posc holds exp(pos): 0 on masked pad-k rows, 1 on pad-q cols;
        # layout [p, (hp, kc, h%2, q)] with k = 128*kc + p
        posc = np.zeros((H, KPAD, NQ), BF16_NP)
        if nk > 0:
            posc[:, :nk, :] = 1.0
            posc[:, :nk, :nq] = np.exp(args["pos_enc"][:, qs:qe, ks:ke]) \
                .swapaxes(1, 2).astype(BF16_NP)
        posm = np.ascontiguousarray(
            posc.reshape(H // 2, 2, NKC, 128, NQ).transpose(3, 0, 1, 2, 4)
            .reshape(128, -1))

        m = {"qfT": feat_tiles(qfc), "kfT": feat_tiles(kfc),
             "vfT": feat_tiles(vfc), "posc": posm,
             "wq": wq8, "wk": wkb, "wv": wvb, "wo": wob}
        if has_bq:
            m["bq"] = (args["bq"] / SCALE).astype(BF16_NP).reshape(1, OD)
        if has_bk:
            m["bk"] = args["bk"].astype(BF16_NP).reshape(1, OD)
        if has_bv:
            m["bv"] = args["bv"].astype(BF16_NP).reshape(1, OD)
        if has_bo:
            m["bo"] = np.ascontiguousarray(
                args["bo"].astype(np.float32).reshape(OD // 128, 128).T)
        in_maps.append(m)

    res = run_bass_kernel_spmd(nc, in_maps, core_ids=list(range(NCORES)),
                               trace=TRACE)
    LAST_RESULTS = res
    out = np.empty((N, OD), np.float32)
    for c in range(NCORES):
        qs, qe = int(qbound[c]), int(qbound[c + 1])
        if qe > qs:
            out[qs:qe, :] = res.results[c]["out"][:, :qe - qs].T.astype(np.float32)
    return out
